# revision 13
# baseline (speedup 1.0000x reference)
"""Single attention head (B=8, S=2048, D=768, H=12) on 8 TRN2 NeuronCores.

Data-parallel over batch (1 element/core). v3 design:
  - Host prep is layout only: per-batch permutation packing masked-in keys
    first (key extent compacts 2048 -> T_pad ~ 1152), x transposed to
    [128, chunk, ko, 512] fp32 for contiguous DMA, weights packed
    [Wk | Wq/sqrt(H) | Wv] fp32 at 32-aligned columns, additive bias row.
  - QKV projection in ONE fp32r pass (fp32r matmuls stream at fp16 rate for
    moving dims >= 256, ~1.5e-4 relative error, fine for this near-one-hot
    softmax; measured end-to-end rel err ~8e-3 vs 2e-2 budget).
  - Pass A (row max, [s,t]): f32r 13-row matmul per s-tile straight from
    the same q/k tiles pass B uses, DVE reduce_max per 512-slab. The max
    row only needs +-11 so fp16-rounded maxes are fine (the shared offset
    cancels between softmax numerator and denominator).
  - Pass B ([t,s]): f32r 14-row matmul (12 q + bias row + "-max" row);
    t-tile pairs share a [128,1024] PSUM tile so ACT exp runs 1024 wide.
  - PV: fp16, column-tiled 2 ways (M=16 at array cols 0/64); denominator
    rides along as a ones-column; DVE adds the two column-group partials.
  - Tile-granular dependency tracking forced per-chunk x / rhs-q tiles so
    DMA streams overlap compute; dummy matmuls keep the PE HAM-warm
    through the DMA-bound head; A-matmuls interleave with B/PV pairs.
"""

import math
import os

import numpy as np

B, S, D, H = 8, 2048, 768, 12
N_CORES = 8
NCH = 4            # s chunks
SCH = S // NCH     # 512
BIAS_B = -1.0e8    # fp32 additive mask bias


def _build(nc_mod, T_pad):
    bass, mybir, tile, bacc = nc_mod
    f32 = mybir.dt.float32
    f32r = mybir.dt.float32r
    f16 = mybir.dt.float16
    AF = mybir.ActivationFunctionType
    X = mybir.AxisListType.X

    NT = T_pad // 128
    slabs = [(o, min(512, T_pad - o)) for o in range(0, T_pad, 512)]
    NSL = len(slabs)
    last_cov = (T_pad - 1) // SCH

    nc = bacc.Bacc("TRN2", target_bir_lowering=False, debug=False,
                   num_devices=N_CORES)

    x_ext = nc.dram_tensor("x", [128, NCH * 6 * SCH], f32r,
                           kind="ExternalInput")
    w_ext = nc.dram_tensor("w", [128, 6 * 76], f32r, kind="ExternalInput")
    onesT_ext = nc.dram_tensor("onesT", [1, T_pad], f16, kind="ExternalInput")
    constB_ext = nc.dram_tensor("constB", [2, T_pad], f32r,
                                kind="ExternalInput")
    onesS_ext = nc.dram_tensor("onesS", [1, S], f32r, kind="ExternalInput")
    out_ext = nc.dram_tensor("out", [128, 256], f32, kind="ExternalOutput")

    from concourse.masks import make_identity

    with tile.TileContext(nc) as tc:
        with tc.tile_pool(name="sb", bufs=1) as sb, \
             tc.tile_pool(name="pp", bufs=3) as ppool, \
             tc.tile_pool(name="qkvp", bufs=1, space="PSUM") as qkvp, \
             tc.tile_pool(name="ap", bufs=2, space="PSUM") as ap, \
             tc.tile_pool(name="bp", bufs=2, space="PSUM") as bp, \
             tc.tile_pool(name="vp", bufs=1, space="PSUM") as vp:

            xc = [sb.tile([128, 6, SCH], f32r, name=f"xc{c}")
                  for c in range(NCH)]
            w = sb.tile([128, 6, 76], f32r)
            kTb = sb.tile([16, T_pad], f32r)    # 0-11 k, 12 bias, 13 = -1
            # per-chunk q tiles: 0-11 q, 12 = 1, 13 = m (written by negm)
            rq = [sb.tile([16, SCH], f32r, name=f"rq{c}")
                  for c in range(NCH)]
            vaugT = sb.tile([32, T_pad], f16)   # 0-11 v, 12 = 1, rest 0
            vaug = sb.tile([128, NT, 16], f16)
            ident = sb.tile([128, 128], f32)
            ident16 = sb.tile([16, 16], f16)
            maxh = sb.tile([128, 16, 4], f32)
            maxc = sb.tile([128, 16], f32)
            negmT = sb.tile([4, 128], f32r)
            vcomb = sb.tile([32, S], f16)       # 0-12 combined out+denom
            vstage = sb.tile([16, S], f32)
            rec4 = sb.tile([128, 16], f32)
            outsb = sb.tile([128, 16, 16], f32)

            nc.gpsimd.memset(vaugT[:, :], 0.0)    # rows 13-31 stay 0
            nc.gpsimd.memset(vcomb[:, :], 0.0)    # rows 13-31 stay 0
            make_identity(nc, ident[:])
            make_identity(nc, ident16[:])

            nc.sync.dma_start(w[:], w_ext.ap().rearrange(
                "p (ko m) -> p ko m", ko=6))
            nc.sync.dma_start(kTb[12:14, :], constB_ext.ap())
            nc.sync.dma_start(vaugT[12:13, :], onesT_ext.ap())
            for c in range(NCH):
                nc.sync.dma_start(rq[c][12:13, :],
                                  onesS_ext.ap()[:, c * SCH:(c + 1) * SCH])
            xr = x_ext.ap().rearrange("p (c ko s) -> p c ko s", c=NCH, ko=6)

            # ---- pass A / negm emitters ----
            def emit_A_st(st):
                c, stl = st // 4, st % 4
                s0 = stl * 128
                for si, (to, tw) in enumerate(slabs):
                    at = ap.tile([128, 512], f32, tag="pa")
                    nc.tensor.matmul(
                        at[:, 0:tw], rq[c][0:13, s0:s0 + 128],
                        kTb[0:13, to:to + tw], start=True, stop=True)
                    nc.vector.reduce_max(
                        maxh[:, st, si:si + 1], at[:, 0:tw], axis=X)
                nc.vector.reduce_max(
                    maxc[:, st:st + 1], maxh[:, st, 0:NSL], axis=X)

            def emit_negm(c):
                c4 = slice(4 * c, 4 * c + 4)
                mt = ap.tile([128, 512], f32, tag="pa")
                nc.tensor.transpose(mt[0:4, 0:128], maxc[:, c4], ident[:])
                nc.scalar.copy(negmT[:, :], mt[0:4, 0:128])
                for k in range(4):
                    nc.sync.dma_start(rq[c][13:14, k * 128:(k + 1) * 128],
                                      negmT[k:k + 1, :])

            # ---- QKV projection (fp32r), one pass, DMA interleaved ----
            for c in range(NCH):
                nc.sync.dma_start(xc[c][:], xr[:, c])
                # dummy matmuls keep the PE HAM-warm through the DMA wait
                nd = 6 if c > 0 else 2
                for dummy in range(nd):
                    dt_ = qkvp.tile([76, SCH], f32, tag="qkv",
                                    name=f"dum{c}_{dummy}")
                    nc.tensor.matmul(dt_[:, :], w[:, 0, :],
                                     xc[max(0, c - 1)][:, 0, :],
                                     start=True, stop=True)
                qkv = qkvp.tile([76, SCH], f32, tag="qkv")
                for ko in range(6):
                    nc.tensor.matmul(qkv[:, :], w[:, ko, :], xc[c][:, ko, :],
                                     start=(ko == 0), stop=(ko == 5))
                nc.scalar.copy(rq[c][0:12, :], qkv[32:44, :])
                if c * SCH < T_pad:
                    t0 = c * SCH
                    t1 = min((c + 1) * SCH, T_pad)
                    tsl = slice(0, t1 - t0)
                    ts = slice(t0, t1)
                    nc.scalar.copy(kTb[0:12, ts], qkv[0:12, tsl])
                    nc.scalar.copy(vaugT[0:12, ts], qkv[64:76, tsl])
                if c == last_cov:
                    for j in range(NT):
                        vt = ap.tile([128, 512], f16, name=f"vt{j}",
                                     tag="pa")
                        nc.tensor.transpose(
                            vt[:, 0:16], vaugT[0:16, j * 128:(j + 1) * 128],
                            ident16[:])
                        nc.vector.tensor_copy(vaug[:, j, 0:16], vt[:, 0:16])
                    for st in range(4):
                        emit_A_st(st)
                    emit_negm(0)
            # pass A for chunk 1 during the QKV tail
            for st in range(4, 8):
                emit_A_st(st)
            emit_negm(1)

            # ---- attention main loop ----
            npair = (NT + 1) // 2
            g_last = {0: 2 * (npair - 1)}
            g_last[1] = 2 * ((NT - 2) // 2) + 1 if NT >= 2 else -1

            def emit_out(c):
                for stl in range(4):
                    st = 4 * c + stl
                    ot = ap.tile([128, 512], f16, name=f"ot{st}", tag="pa")
                    nc.tensor.transpose(
                        ot[:, 0:16], vcomb[0:16, st * 128:(st + 1) * 128],
                        ident16[:])
                    nc.vector.reciprocal(rec4[:, st:st + 1], ot[:, 12:13])
                    nc.vector.tensor_scalar_mul(
                        outsb[:, st, 0:12], ot[:, 0:12], rec4[:, st:st + 1])

            for c in range(NCH):
                cs = slice(c * SCH, (c + 1) * SCH)
                vacc = vp.tile([96, SCH], f32, tag="v")
                for jp in range(npair):
                    j0, j1 = 2 * jp, 2 * jp + 1
                    width = 1024 if j1 < NT else 512
                    bt = bp.tile([128, 1024], f32, tag="b")
                    nc.tensor.matmul(
                        bt[:, 0:512], kTb[0:14, j0 * 128:(j0 + 1) * 128],
                        rq[c][0:14, :], start=True, stop=True)
                    if j1 < NT:
                        nc.tensor.matmul(
                            bt[:, 512:1024],
                            kTb[0:14, j1 * 128:(j1 + 1) * 128],
                            rq[c][0:14, :], start=True, stop=True)
                    p = ppool.tile([128, 1024], f16, tag="p")
                    nc.scalar.activation(p[:, 0:width], bt[:, 0:width],
                                         AF.Exp)
                    nc.tensor.matmul(
                        vacc[0:16, :], vaug[:, j0, 0:16], p[:, 0:512],
                        start=(j0 == 0), stop=(j0 == g_last[0]),
                        tile_position=(0, 0))
                    if j1 < NT:
                        nc.tensor.matmul(
                            vacc[64:80, :], vaug[:, j1, 0:16],
                            p[:, 512:1024],
                            start=(j1 == 1), stop=(j1 == g_last[1]),
                            tile_position=(0, 64))
                    # interleave next-next chunk's pass A between B pairs
                    if c + 2 < NCH and jp < 4:
                        emit_A_st(4 * (c + 2) + jp)
                nc.scalar.copy(vstage[0:16, cs], vacc[64:80, :])
                nc.vector.tensor_add(vcomb[0:16, cs], vacc[0:16, :],
                                     vstage[0:16, cs])
                if c + 2 < NCH:
                    emit_negm(c + 2)
                emit_out(c)

            nc.sync.dma_start(
                out_ext.ap(), outsb[:].rearrange("p a b -> p (a b)"))

    nc.compile()
    return nc


def kernel(x, mask, key_weight, query_weight, value_weight):
    import concourse.bass as bass
    import concourse.mybir as mybir
    import concourse.tile as tile
    from concourse import bacc, bass_utils

    x = np.asarray(x, dtype=np.float32)
    mask = np.asarray(mask)
    wk = np.asarray(key_weight, dtype=np.float32)
    wq = np.asarray(query_weight, dtype=np.float32)
    wv = np.asarray(value_weight, dtype=np.float32)

    w2 = np.zeros((D, 76), dtype=np.float32)
    w2[:, 0:12] = wk
    w2[:, 32:44] = wq / math.sqrt(H)
    w2[:, 64:76] = wv
    w_dev = np.ascontiguousarray(
        w2.reshape(6, 128, 76).transpose(1, 0, 2)).reshape(128, 6 * 76)

    perms, nbs = [], []
    for b in range(B):
        m = mask[b, 0].astype(np.int64)
        perm = np.argsort(1 - m, kind="stable")
        perms.append(perm)
        nbs.append(int(m.sum()))
    T_pad = max(128, int(np.ceil(max(max(nbs), 1) / 128.0)) * 128)
    T_pad = min(T_pad, S)

    in_maps = []
    for b in range(B):
        xp = x[b][perms[b]]                    # [S, D]
        xp = xp.reshape(NCH, SCH, 6, 128)      # [c, s, ko, p]
        x_dev = np.ascontiguousarray(
            xp.transpose(3, 0, 2, 1)).reshape(128, NCH * 6 * SCH)
        constB = np.zeros((2, T_pad), dtype=np.float32)
        constB[0, nbs[b]:] = BIAS_B
        constB[1, :] = -1.0
        in_maps.append({"x": x_dev, "w": w_dev,
                        "constB": constB,
                        "onesS": np.ones((1, S), dtype=np.float32),
                        "onesT": np.ones((1, T_pad), dtype=np.float16)})

    import time as _time
    _t0 = _time.time()
    print(f"[kernel] building graph, T_pad={T_pad}", flush=True)
    nc = _build((bass, mybir, tile, bacc), T_pad)
    print(f"[kernel] graph+bacc compile done in {_time.time() - _t0:.1f}s",
          flush=True)

    trace = os.environ.get("BASS_KERNEL_TRACE", "0") == "1"
    if trace:
        import sys
        import types
        from trn_agent_boot.trn_boot import _ntff_profile_via_ctypes
        hook = _ntff_profile_via_ctypes("/opt/axon/libaxon_pjrt.so")
        m = types.ModuleType("antenv.axon_hooks")
        m.get_axon_ntff_profile_hook = lambda: hook
        sys.modules["antenv.axon_hooks"] = m
        bass_utils.upload_artifacts = lambda tmpdir: "local://" + tmpdir

    res = bass_utils.run_bass_kernel_spmd(
        nc, in_maps, core_ids=list(range(N_CORES)), trace=trace)
    if trace:
        print(f"HW exec time: {res.exec_time_ns} ns", flush=True)

    out = np.empty((B, S, H), dtype=np.float32)
    for b in range(B):
        o = res.results[b]["out"].reshape(128, 16, 16)[:, :, :H]
        out[b, perms[b], :] = o.transpose(1, 0, 2).reshape(S, H)
    return out
